# revision 65
# baseline (speedup 1.0000x reference)
"""LeViT-style attention block kernel for Trainium2 (8 NeuronCores, data-parallel over batch).

Reference computation (per batch b of 256, N=196 tokens, DIM=384):
  qkv = x @ qkv_w.T + qkv_b                      [196, 1152]
  q,k,v per head h (6): q,k [196,32], v [196,128]
  S = q @ k.T * 32^-0.5 + bias_h                 [196, 196]
  P = softmax(S, -1)
  O = P @ v  (concat heads -> [196, 768])
  A = hardswish(O)
  out = A @ proj_w.T + proj_b                    [196, 384]

Device mapping (per core: 32 batches = 6272 tokens):
  - host prepacks x.T (bf16), weight tiles, exp(bias)^T tables
  - q/k computed transposed ([head-packed 128 rows, tokens]) on PE,
    interleaved into the batch loop (2 chunks of runway) as PE filler
  - v computed natural per batch ([tokens, 768])
  - S^T[m,n] = k^T q directly on PE (keys m on partitions, queries n free)
    -> exp on ACT -> *exp(bias)^T on DVE = P-hat^T
    -> den[n] = all-ones matmul over P-hat^T rows (PE; [msz,128] ones
       stationary broadcasts den to all partitions), rho = 1/den via
       reciprocal_approx_fast on DVE
    -> O^T = v^T @ P-hat^T on PE (v natural is already the right stationary)
    -> normalize O^T by rho + hardswish on DVE; v-bias of heads 3-5 folded
       here per-partition (softmax rows sum to 1, so O = P(xWv)/den + bv)
    -> A^T per head -> proj matmul (lagged one batch as PE filler) -> out.
  No transpose matmuls and no GpSimd work (the v1 kernel spent 70% of the
  span in GpSimd diag builds for a P^T-via-matmul transpose: 885us).

Optimizations over the 341us baseline (measures ~280us NTFF HW exec on a
cool device; the shared trn2 chip sometimes sits in a P0 2.0GHz state
where everything is uniformly ~19% slower):
  - proj computed transposed (out^T = W @ A^T) over batch PAIRS (N=392):
    3 full 128-row c-tiles instead of 128+68 token tiles, no 1-partition
    bias matmuls (proj bias rides the ACT evacuation per-partition),
    output [DIM, T] transposed back on host.
  - v-bias applied per-partition in O^T layout for ALL heads (vbc tensor
    _scalar adds); both v halves evacuate via ACT Copy.
  - q4/q5/k4/k5 packed into ONE qkv dest (3 full-width matmuls per chunk
    instead of 2 full + 2 half); k45 copied to rows 0:64 of a shadow tile
    by an SBUF->SBUF DMA (walrus requires lhsT/rhs partition bases match).
  - all 12 S^T matmuls emitted in one wave (4 tile_position row strips
    pipeline 3 matmuls each); ot_ps shares the sps PSUM slots (same
    shape+tag) so 3 s2 tiles + 2 mm banks fit the 8 PSUM banks.
  - xt split into 9 independently-DMA'd tiles, all inputs host-packed in
    exact SBUF layouts (straight large-descriptor DMA copies), ordered by
    first use; PE warmup matmuls trip the HAM clock gate during the
    initial DMA wait.
  - qkv chunks spread to batch bc-2 (QKV_AHEAD=1) so tail batches keep PE
    filler (prevents end-of-kernel HAM re-throttle); final pair's proj
    split per-batch to shorten the drain.
"""

import os
import sys

import numpy as np

sys.path.insert(0, "/opt/trn_rl_repo")

import ml_dtypes  # noqa: E402

import concourse.bass as bass  # noqa: E402
import concourse.tile as tile  # noqa: E402
from concourse import bacc, mybir  # noqa: E402
from concourse.bass_utils import run_bass_kernel_spmd  # noqa: E402

BF16 = mybir.dt.bfloat16
F32 = mybir.dt.float32
F8 = mybir.dt.float8e4
NPBF16 = ml_dtypes.bfloat16
NPF8 = mybir.dt.np(F8)
DR = mybir.MatmulPerfMode.DoubleRow

N_CORES = 8
B, N, DIM = 256, 196, 384
KD, NH, D = 32, 6, 128  # key dim, heads, per-head v dim
DH = D * NH  # 768
RES = 14
SCALE = KD ** -0.5

AF = mybir.ActivationFunctionType
OP = mybir.AluOpType

# per-batch key/query token tiling: 196 rows split 128 + 68
NT = [(0, 128), (128, 68)]

LAST_RESULT = {}  # test harness peeks at timing info here

# head-pairs whose attention bias is added into S on the PE (identity
# matmul accumulate) instead of multiplied as exp(bias) on the DVE —
# a balance knob between the two engines (0..3)
BADD_PAIRS = 0
# pack q4,q5,k4,k5 into ONE qkv dest tile (rows 0:32/32:64/64:96/96:128),
# saving 3 half-empty matmuls per chunk; S matmuls for heads 4-5 then use
# stationary/moving APs with different partition bases + explicit
# tile_position
QK_PACK = True


def _build_nc(bc):
    """Build the single-core program for bc batches (bc*196 tokens)."""
    T = bc * N
    assert T % 392 == 0
    nch = T // 392

    nc = bacc.Bacc("TRN2", target_bir_lowering=False, debug=False, num_devices=N_CORES)

    xt_d = nc.dram_tensor("xt", [8, 128, 3, T // 8], BF16, kind="ExternalInput")
    wqk_d = nc.dram_tensor("wqk", [128, 4, 3, 128], BF16, kind="ExternalInput")
    bqk_d = nc.dram_tensor("bqk", [128, 4], F32, kind="ExternalInput")
    wv_d = nc.dram_tensor("wv", [128, 3, DH], BF16, kind="ExternalInput")
    vb_d = nc.dram_tensor("vb", [128, DH], F32, kind="ExternalInput")
    vbc_d = nc.dram_tensor("vbc", [128, NH], F32, kind="ExternalInput")
    wpt_d = nc.dram_tensor("wpt", [128, NH, 3, 128], BF16, kind="ExternalInput")
    pbc_d = nc.dram_tensor("pbc", [128, 3], F32, kind="ExternalInput")
    onem_d = nc.dram_tensor("onem", [128, 128], BF16, kind="ExternalInput")
    ebt_d = nc.dram_tensor("ebt", [128, NH // 2, 1024], BF16, kind="ExternalInput")
    btt_d = nc.dram_tensor("btt", [128, NH // 2, 1024], BF16, kind="ExternalInput")
    idm_d = nc.dram_tensor("idm", [128, 128], BF16, kind="ExternalInput")
    out_d = nc.dram_tensor("out", [DIM, T], F32, kind="ExternalOutput")

    with tile.TileContext(nc) as tc:
        with (  # noqa: SIM117
            tc.tile_pool(name="const", bufs=1) as cpool,
            tc.tile_pool(name="qkt", bufs=1) as qpool,
            tc.tile_pool(name="vtile", bufs=4) as vpool,
            tc.tile_pool(name="pu", bufs=3) as pupool,
            tc.tile_pool(name="pb2", bufs=4) as pbpool,
            tc.tile_pool(name="rho", bufs=4) as rpool,
            tc.tile_pool(name="onm", bufs=3) as onpool,
            tc.tile_pool(name="atc", bufs=3) as atpool,
            tc.tile_pool(name="ob", bufs=3) as opool,
            tc.tile_pool(name="mmps", bufs=2, space="PSUM") as mmps,
            tc.tile_pool(name="sps", bufs=3, space="PSUM") as sps,
        ):
            # ---- constants into SBUF, ordered by first use so the first
            #      qkv chunk (wqk+xq0) isn't queued behind later-needed
            #      megabyte-scale DMAs. xt is split into 8 independent
            #      tiles (4 batches each) so early matmuls only wait on
            #      ~0.6MB of the 4.8MB input stream ----
            XQ = T // 8  # 784 tokens = 4 batches per xt tile
            # tile 0 split in half again so the very first qkv chunk (392
            # tokens) only waits on ~0.3MB
            xt_q = [
                cpool.tile([128, 3, 392], BF16, tag=f"xq0{i}", name=f"xq0{i}")
                for i in range(2)
            ] + [
                cpool.tile([128, 3, XQ], BF16, tag=f"xq{i}", name=f"xq{i}")
                for i in range(1, 8)
            ]
            # critical first-compute tensors split across BOTH DMA queues
            # so they land in parallel: wqk on sync, xq0a on scalar; onem
            # first (tiny) to feed the PE warmup matmuls below
            onem_t = cpool.tile([128, 128], BF16)
            nc.sync.dma_start(onem_t[:], onem_d[:])
            wqk_t = cpool.tile([128, 4, 3, 128], BF16)
            nc.sync.dma_start(wqk_t[:], wqk_d[:])
            nc.sync.dma_start(xt_q[0][:, :, :], xt_d[0, :, :, 0:392])
            nc.sync.dma_start(xt_q[1][:, :, :], xt_d[0, :, :, 392:784])
            bqk_t = cpool.tile([128, 4], F32)
            nc.sync.dma_start(bqk_t[:], bqk_d[:])
            wv_t = cpool.tile([128, 3, DH], BF16)
            nc.scalar.dma_start(wv_t[:], wv_d[:])
            vbc_t = cpool.tile([128, NH], F32)
            nc.sync.dma_start(vbc_t[:], vbc_d[:])
            ebt_t = cpool.tile([128, NH // 2, 1024], BF16)
            nc.scalar.dma_start(ebt_t[:], ebt_d[:])
            nc.sync.dma_start(xt_q[2][:, :, :], xt_d[1])
            if BADD_PAIRS:
                btt_t = cpool.tile([128, NH // 2, 1024], BF16)
                nc.scalar.dma_start(btt_t[:], btt_d[:])
                idm_t = cpool.tile([128, 128], BF16)
                nc.scalar.dma_start(idm_t[:], idm_d[:])
            pbc_t = cpool.tile([128, 3], F32)
            nc.scalar.dma_start(pbc_t[:], pbc_d[:])
            wpt_t = cpool.tile([128, NH, 3, 128], BF16)
            nc.scalar.dma_start(wpt_t[:], wpt_d[:])
            for i in range(2, 8):
                nc.sync.dma_start(xt_q[i + 1][:, :, :], xt_d[i])

            def xt_at(t0):
                """(tile, col offset) for global token t0 (range stays in-tile)."""
                if t0 < 784:
                    return xt_q[t0 // 392], t0 % 392
                return xt_q[2 + (t0 - 784) // XQ], (t0 - 784) % XQ

            # ~3.5us of dummy matmuls (onem x onem -> scratch PSUM) fill the
            # otherwise-idle PE during the initial input DMA, tripping the
            # HAM activity window so the first real matmuls run at 2.4GHz
            # instead of cold 1.2GHz. Rotating target regions lets them
            # pipeline back-to-back instead of serializing on WAW.
            warm_ps = mmps.tile([128, 392], F32, tag="mm", name="warm_ps")
            for i in range(32):
                r0 = 128 * (i % 3)
                nc.tensor.matmul(
                    warm_ps[:, r0:r0 + 128], onem_t[:, :], onem_t[:, :],
                    start=True, stop=True)

            # ---- q/k^T phase: tQ/tK hold q/k of heads 0-3 at partition 32h;
            #      heads 4-5 either packed in one tile (q4,q5,k4,k5 at rows
            #      0/32/64/96) or split across tQ2/tK2 (rows 0:64) ----
            tQ = qpool.tile([128, T], BF16, tag="tq")
            tK = qpool.tile([128, T], BF16, tag="tk")
            if QK_PACK:
                tQK2 = qpool.tile([128, T], BF16, tag="tqk2")
                # k4/k5 land at rows 64:128 of tQK2; a SBUF->SBUF DMA per
                # chunk copies them down to rows 0:64 of tK2c so the S
                # matmuls see matched q/k partition bases (walrus requires
                # lhsT.base == rhs.base)
                tK2c = qpool.tile([64, T], BF16, tag="tk2c")
                qk_dests = [(0, tQ, 128), (1, tK, 128), (2, tQK2, 128)]
            else:
                tQ2 = qpool.tile([128, T], BF16, tag="tq2")
                tK2 = qpool.tile([128, T], BF16, tag="tk2")
                qk_dests = [
                    (0, tQ, 128), (1, tK, 128), (2, tQ2, 64), (3, tK2, 64)]

            def emit_qkv_chunk(ch):
                c0 = 392 * ch
                xq, xo = xt_at(c0)
                for mt, dest, msz in qk_dests:
                    ps = mmps.tile([128, 392], F32, tag="mm")
                    for ct in range(3):
                        nc.tensor.matmul(
                            ps[0:msz, :],
                            wqk_t[:, mt, ct, 0:msz],
                            xq[:, ct, xo:xo + 392],
                            start=(ct == 0), stop=(ct == 2),
                        )
                    nc.scalar.activation(
                        dest[0:msz, c0:c0 + 392], ps[0:msz, :], AF.Identity,
                        bias=bqk_t[0:msz, mt:mt + 1], scale=1.0,
                    )
                if QK_PACK:
                    nc.sync.dma_start(
                        tK2c[0:64, c0:c0 + 392], tQK2[64:128, c0:c0 + 392])

            # chunk c feeds batches 2c and 2c+1; AHEAD chunks of runway up
            # front, the rest interleaved into the batch loop as PE filler.
            # AHEAD=1 keeps chunk emissions flowing through batch bc-2 so
            # the tail batches still have PE filler (prevents end-of-kernel
            # HAM re-throttle)
            QKV_AHEAD = 1
            for ch in range(QKV_AHEAD):
                emit_qkv_chunk(ch)

            def emit_s(hp, b0):
                """S^T matmuls for one head pair; head hh at column 512*hh of a
                bank-padded [128,1024] PSUM tile (m-tile mt at 512*hh+196*mt).
                Pairs 0 and 1 are emitted back-to-back so their 4 distinct
                tile_position row groups can run concurrently in the PE."""
                s2 = sps.tile([128, 1024], F32, tag="s")
                badd = hp < BADD_PAIRS
                if badd:
                    # seed the PSUM with the attention bias (id.T @ B^T = B^T);
                    # the S matmuls then accumulate on top (start=False)
                    for hh in range(2):
                        nc.tensor.matmul(
                            s2[:, 512 * hh:512 * hh + 392],
                            idm_t[:, :],
                            btt_t[:, hp, 512 * hh:512 * hh + 392],
                            start=True, stop=False,
                        )
                # mt-major order so consecutive matmuls hit different
                # tile_position row groups (they overlap in the array)
                for mt, (m0, msz) in enumerate(NT):
                    for hh in range(2):
                        h = 2 * hp + hh
                        if h < 4:
                            qb = kb = tp = 32 * h
                            qsrc = tQ
                            ksrc = tK
                        elif QK_PACK:
                            # q at rows 0:64 of tQK2, k copied to rows
                            # 0:64 of tK2c (matched partition bases)
                            qb = kb = tp = 32 * (h - 4)
                            qsrc, ksrc = tQK2, tK2c
                        else:
                            qb = kb = tp = 32 * (h - 4)
                            qsrc, ksrc = tQ2, tK2
                        nc.tensor.matmul(
                            s2[0:msz, 512 * hh + 196 * mt:512 * hh + 196 * mt + 196],
                            ksrc[kb:kb + 32, b0 + m0:b0 + m0 + msz],
                            qsrc[qb:qb + 32, b0:b0 + 196],
                            start=not badd, stop=True,
                            tile_position=(tp, 0),
                        )
                return s2

            def pair_exp(hp, s2):
                """ACT exp (+DVE bias-mul) for one head pair -> P-hat^T.
                Emitted for all three pairs right after the S wave so the
                exps sit at the FRONT of the ACT FIFO (they gate each
                pair's den/recip/OT chain; the v/filler evacuations can
                wait)."""
                s2v = s2[:].rearrange("p (h c) -> p h c", h=2)[:, :, 0:392]
                pb2 = pbpool.tile([128, 1024], BF16, tag="pb")
                pbv = pb2[:].rearrange("p (h c) -> p h c", h=2)[:, :, 0:392]
                if hp < BADD_PAIRS:
                    # bias already in S (PE-seeded): P-hat^T = exp(S^T+B^T)
                    nc.scalar.activation(pbv, s2v, AF.Exp)
                else:
                    # P-hat^T = exp(S^T) * exp(bias)^T
                    p_u = pupool.tile([128, 1024], BF16, tag="pu")
                    p_uv = p_u[:].rearrange("p (h c) -> p h c", h=2)[:, :, 0:392]
                    nc.scalar.activation(p_uv, s2v, AF.Exp)
                    ebv = ebt_t[:, hp, :].rearrange(
                        "p (h c) -> p h c", h=2)[:, :, 0:392]
                    nc.vector.tensor_mul(pbv, p_uv, ebv)
                return pb2

            def process_pair(hp, pb2, v_t, at2, par):
                # strided [128, 2, 392] views skip the 392:512 pad columns
                # (pads are never read downstream — den/OT use exact regions)
                # den[n] = sum_m P-hat^T[m, n] via all-ones matmul; the
                # [msz,128] ones stationary broadcasts den to all 128
                # partitions so the normalize multiply needs no
                # partition-broadcast AP (DVE requires nonzero step).
                # One matmul per m-tile covers BOTH heads via a strided
                # rhs AP [msz, 2, 196] (head stride 512).
                # den lands in ot_ps first; the O^T matmuls overwrite it
                # after the reciprocal is taken (WAR order via Tile).
                # ot_ps shares the sps pool slots (same shape+tag as s2) so
                # three s2 tiles can be alive at once within 8 PSUM banks;
                # only cols 0:392 are used.
                ot_ps = sps.tile([128, 1024], F32, tag="s", name="ot_ps")
                pb2v = pb2[:].rearrange("p (h c) -> p h c", h=2)
                for mt, (m0, msz) in enumerate(NT):
                    nc.tensor.matmul(
                        ot_ps[:, 0:392],
                        onem_t[0:msz, :],
                        pb2v[0:msz, :, 196 * mt:196 * mt + 196],
                        start=(mt == 0), stop=(mt == 1),
                    )
                rho = rpool.tile([128, 392], F32, tag="rho")
                nc.vector.reciprocal_approx_fast(rho[:, :], ot_ps[:, 0:392])
                # ---- O^T [128, 196] per head = v^T @ P-hat^T ----
                for hh in range(2):
                    h = 2 * hp + hh
                    for mt, (m0, msz) in enumerate(NT):
                        nc.tensor.matmul(
                            ot_ps[:, 196 * hh:196 * hh + 196],
                            v_t[0:msz, mt, 128 * h:128 * h + 128],
                            pb2[0:msz, 512 * hh + 196 * mt:512 * hh + 196 * mt + 196],
                            start=(mt == 0), stop=(mt == 1),
                        )
                # ---- normalize by rho; ALL heads add the v-bias here
                #      (per-partition in O^T layout; softmax rows sum to 1
                #      so O = P(xWv)/den + bv) ----
                o_n = onpool.tile([128, 392], BF16, tag="on")
                nc.vector.tensor_mul(o_n[:, :], ot_ps[:, 0:392], rho[:, :])
                o_nb = onpool.tile([128, 392], BF16, tag="onb")
                for hh in range(2):
                    h = 2 * hp + hh
                    reg = slice(196 * hh, 196 * hh + 196)
                    nc.vector.tensor_scalar_add(
                        o_nb[:, reg], o_n[:, reg], vbc_t[:, h:h + 1])
                # ---- 6*hardswish(O) = O*(clamp(O,-3,3)+3); the /6 is
                #      folded into the projection weights ----
                m_t = onpool.tile([128, 392], BF16, tag="mt")
                nc.vector.tensor_scalar(
                    out=m_t[:, :], in0=o_nb[:, :],
                    scalar1=3.0, scalar2=-3.0, op0=OP.min, op1=OP.max,
                )
                # at2 is [128, par, head, 196]; this write (par, 2hp..2hp+2)
                # is one contiguous [128, 392] span, keeping DVE in 2x mode
                # (scalar_tensor_tensor only runs at 1x — measured slower
                # than this TS+TT pair)
                m3 = onpool.tile([128, 392], BF16, tag="m3")
                nc.vector.tensor_scalar_add(m3[:, :], m_t[:, :], 3.0)
                nc.vector.tensor_mul(
                    at2[:, par, 2 * hp:2 * hp + 2, :], m3[:].rearrange(
                        "p (h c) -> p h c", h=2),
                    o_nb[:].rearrange("p (h c) -> p h c", h=2))

            def emit_proj(p, at2):
                # out^T [384, 392] for batch pair p: 3 ctiles x 6 head-ktiles,
                # proj bias added per-partition during the ACT evacuation
                c0 = 392 * p
                for ct in range(3):
                    ps = mmps.tile([128, 392], F32, tag="mm")
                    for kt in range(NH):
                        nc.tensor.matmul(
                            ps[:, :], wpt_t[:, kt, ct, :], at2[:, :, kt, :],
                            start=(kt == 0), stop=(kt == NH - 1),
                        )
                    ob = opool.tile([128, 392], F32, tag="ob")
                    nc.scalar.activation(
                        ob[:, :], ps[:, :], AF.Identity,
                        bias=pbc_t[:, ct:ct + 1], scale=1.0)
                    nc.sync.dma_start(
                        out_d[128 * ct:128 * ct + 128, c0:c0 + 392], ob[:, :])

            def emit_proj_half(p, at2, par):
                # one batch's worth of proj (N=196) — used to pull half of
                # the final pair's projection off the kernel tail
                c0 = 392 * p + 196 * par
                for ct in range(3):
                    ps = mmps.tile([128, 392], F32, tag="mm")
                    for kt in range(NH):
                        nc.tensor.matmul(
                            ps[:, 0:196], wpt_t[:, kt, ct, :],
                            at2[:, par, kt, :],
                            start=(kt == 0), stop=(kt == NH - 1),
                        )
                    ob = opool.tile([128, 392], F32, tag="ob")
                    nc.scalar.activation(
                        ob[:, 0:196], ps[:, 0:196], AF.Identity,
                        bias=pbc_t[:, ct:ct + 1], scale=1.0)
                    nc.sync.dma_start(
                        out_d[128 * ct:128 * ct + 128, c0:c0 + 196],
                        ob[:, 0:196])

            for b in range(bc):
                b0 = b * N
                # ---- v natural [tokens, 768] for this batch; low half
                #      evacuated on DVE (with bias add), high half on ACT
                #      (bias for heads 3-5 folded into o_n above) ----
                v_t = vpool.tile([128, 2, DH], BF16, tag="v")
                xq, xo = xt_at(b0)
                # half-major so the pair-0 O^T matmuls (heads 0-1, half 0)
                # unblock after the first two evacuations
                for half in range(2):
                    h0 = 384 * half
                    for nt, (r0, nsz) in enumerate(NT):
                        ps = mmps.tile([128, 392], F32, tag="mm")
                        for ct in range(3):
                            nc.tensor.matmul(
                                ps[0:nsz, 0:384],
                                xq[:, ct, xo + r0:xo + r0 + nsz],
                                wv_t[:, ct, h0:h0 + 384],
                                start=(ct == 0), stop=(ct == 2),
                            )
                        nc.scalar.activation(
                            v_t[0:nsz, nt, h0:h0 + 384], ps[0:nsz, 0:384],
                            AF.Copy)

                par = b % 2
                if par == 0:
                    if b > 0:
                        done_at2 = at2  # pair b//2-1 fully written
                    at2 = atpool.tile([128, 2, NH, 196], BF16, tag="at")
                # all 12 S matmuls in one wave: the 4 tile_position row
                # strips pipeline 3 matmuls each; the three exps are
                # emitted immediately so they sit at the FRONT of the ACT
                # FIFO (ahead of the v/filler evacuations they don't need)
                s2a = emit_s(0, b0)
                s2b = emit_s(1, b0)
                s2c = emit_s(2, b0)
                pb_a = pair_exp(0, s2a)
                pb_b = pair_exp(1, s2b)
                pb_c = pair_exp(2, s2c)
                # pair 0's den/OT go ahead of the fillers (its pb2 is ready
                # ~2us into the batch, right as the v matmuls finish)
                process_pair(0, pb_a, v_t, at2, par)
                # PE fillers while pairs 1-2's bias-muls run: qkv chunks
                # on even batches, proj of the previous PAIR on odd batches
                if par == 0 and b // 2 + QKV_AHEAD < nch:
                    emit_qkv_chunk(b // 2 + QKV_AHEAD)
                if par == 1 and b > 1:
                    emit_proj(b // 2 - 1, done_at2)
                    if b == bc - 1:
                        # first half of the final pair's proj runs as
                        # filler here (its par=0 data completed last batch)
                        emit_proj_half(bc // 2 - 1, at2, 0)
                process_pair(1, pb_b, v_t, at2, par)
                process_pair(2, pb_c, v_t, at2, par)
            emit_proj_half(bc // 2 - 1, at2, 1)

    nc.finalize()  # run Bacc passes (reg alloc, wait splitting) before walrus
    return nc


def _host_pack(x, qkv_w, qkv_b, proj_w, proj_b, attn_biases, bias_idxs, bc):
    """Build the common (replicated) input map and per-core xt slices."""
    w = np.asarray(qkv_w, np.float32).reshape(NH, 192, DIM)
    bia = np.asarray(qkv_b, np.float32).reshape(NH, 192)
    qw = w[:, 0:KD, :] * SCALE          # [6, 32, 384]
    kw = w[:, KD:2 * KD, :]
    vw = w[:, 2 * KD:, :]               # [6, 128, 384]
    qb = bia[:, 0:KD] * SCALE
    kb = bia[:, KD:2 * KD]
    vb = bia[:, 2 * KD:]

    wqk = np.zeros((4, DIM, 128), np.float32)
    wqk[0, :, :] = qw[0:4].reshape(128, DIM).T
    wqk[1, :, :] = kw[0:4].reshape(128, DIM).T
    bqk = np.zeros((128, 4), np.float32)
    bqk[:, 0] = qb[0:4].reshape(128)
    bqk[:, 1] = kb[0:4].reshape(128)
    if QK_PACK:
        # q4,q5 at dest rows 0:64, k4,k5 at rows 64:128 of one tile
        wqk[2, :, 0:64] = qw[4:6].reshape(64, DIM).T
        wqk[2, :, 64:128] = kw[4:6].reshape(64, DIM).T
        bqk[0:64, 2] = qb[4:6].reshape(64)
        bqk[64:128, 2] = kb[4:6].reshape(64)
    else:
        wqk[2, :, 0:64] = qw[4:6].reshape(64, DIM).T
        wqk[3, :, 0:64] = kw[4:6].reshape(64, DIM).T
        bqk[0:64, 2] = qb[4:6].reshape(64)
        bqk[0:64, 3] = kb[4:6].reshape(64)

    wv = vw.reshape(DH, DIM).T.copy()          # [384, 768], head h at cols 128h
    vbt = np.tile(vb.reshape(1, DH), (128, 1)).astype(np.float32)
    vbc = np.ascontiguousarray(vb.T)           # [128, 6], col h = head h's bias
    # device computes 6*hardswish; absorb the 1/6 into the projection weights.
    # proj runs transposed (out^T = W @ A^T): wpt[p, kt, ct, c] = W6[128ct+c, 128kt+p]
    w6 = np.asarray(proj_w, np.float32) / 6.0  # [384, 768]
    wpt = np.ascontiguousarray(
        w6.reshape(3, 128, NH, 128).transpose(3, 2, 0, 1))  # [128, 6, 3, 128]
    pbc = np.ascontiguousarray(
        np.asarray(proj_b, np.float32).reshape(3, 128).T)  # [128, 3]

    bmat = np.asarray(attn_biases, np.float32)[:, np.asarray(bias_idxs)]  # [6,196,196]
    ebp = np.zeros((128, NH // 2, 1024), np.float32)  # exp(bias)^T for DVE pairs
    btp = np.zeros((128, NH // 2, 1024), np.float32)  # raw bias^T for PE-add pairs
    bT = np.transpose(bmat, (0, 2, 1))  # [6, m, n]
    ebT = np.exp(bT)
    for h in range(NH):
        hp, hh = divmod(h, 2)
        ebp[0:128, hp, 512 * hh + 0:512 * hh + 196] = ebT[h, 0:128, :]
        ebp[0:68, hp, 512 * hh + 196:512 * hh + 392] = ebT[h, 128:196, :]
        btp[0:128, hp, 512 * hh + 0:512 * hh + 196] = bT[h, 0:128, :]
        btp[0:68, hp, 512 * hh + 196:512 * hh + 392] = bT[h, 128:196, :]

    # DMA-friendly layouts: exactly the SBUF tile layouts, so every input
    # DMA is a straight large-descriptor copy (no gather)
    wqk_p = np.ascontiguousarray(
        wqk.reshape(4, 3, 128, 128).transpose(2, 0, 1, 3))  # [128,4,3,128]
    wv_p = np.ascontiguousarray(
        wv.reshape(3, 128, DH).transpose(1, 0, 2))  # [128,3,768]

    common = {
        "pbc": pbc,
        "btt": btp.astype(NPBF16),
        "idm": np.eye(128, dtype=np.float32).astype(NPBF16),
        "onem": np.ones((128, 128), NPBF16),
        "wqk": wqk_p.astype(NPBF16),
        "bqk": bqk,
        "wv": wv_p.astype(NPBF16),
        "vb": vbt,
        "vbc": vbc.astype(np.float32),
        "wpt": wpt.astype(NPBF16),
        "ebt": ebp.astype(NPBF16),
    }

    x = np.asarray(x, np.float32)
    n_cores = x.shape[0] // bc
    T = bc * N
    xts = []
    for c in range(n_cores):
        xc = x[bc * c:bc * (c + 1)].reshape(T, DIM)
        xt = np.ascontiguousarray(xc.T).astype(NPBF16)  # [384, T]
        xt8 = np.ascontiguousarray(
            xt.reshape(3, 128, 8, T // 8).transpose(2, 1, 0, 3))
        xts.append(xt8)  # [8, 128, 3, T//8]
    return common, xts


_NC_CACHE = {}


def kernel(x, qkv_w, qkv_b, proj_w, proj_b, attn_biases, bias_idxs):
    bc = B // N_CORES
    if bc not in _NC_CACHE:
        _NC_CACHE[bc] = _build_nc(bc)
    nc = _NC_CACHE[bc]
    common, xts = _host_pack(x, qkv_w, qkv_b, proj_w, proj_b, attn_biases, bias_idxs, bc)
    in_maps = [dict(common, xt=xts[c]) for c in range(N_CORES)]
    trace = bool(int(os.environ.get("KT_TRACE", "0")))
    res = run_bass_kernel_spmd(nc, in_maps, list(range(N_CORES)), trace=trace)
    LAST_RESULT["exec_time_ns"] = res.exec_time_ns
    LAST_RESULT["mean_exec_time_ns"] = res.mean_exec_time_ns
    # device emits out^T [DIM, T] per core; transpose back on host
    outs = [
        np.ascontiguousarray(res.results[c]["out"].T).reshape(bc, N, DIM)
        for c in range(N_CORES)
    ]
    return np.concatenate(outs, axis=0).astype(np.float32)



# revision 67
# speedup vs baseline: 1.0481x; 1.0481x over previous
"""LeViT-style attention block kernel for Trainium2 (8 NeuronCores, data-parallel over batch).

Reference computation (per batch b of 256, N=196 tokens, DIM=384):
  qkv = x @ qkv_w.T + qkv_b                      [196, 1152]
  q,k,v per head h (6): q,k [196,32], v [196,128]
  S = q @ k.T * 32^-0.5 + bias_h                 [196, 196]
  P = softmax(S, -1)
  O = P @ v  (concat heads -> [196, 768])
  A = hardswish(O)
  out = A @ proj_w.T + proj_b                    [196, 384]

Device mapping (per core: 32 batches = 6272 tokens):
  - host prepacks x.T (bf16), weight tiles, exp(bias)^T tables
  - q/k computed transposed ([head-packed 128 rows, tokens]) on PE,
    interleaved into the batch loop (2 chunks of runway) as PE filler
  - v computed natural per batch ([tokens, 768])
  - S^T[m,n] = k^T q directly on PE (keys m on partitions, queries n free)
    -> exp on ACT -> *exp(bias)^T on DVE = P-hat^T
    -> den[n] = all-ones matmul over P-hat^T rows (PE; [msz,128] ones
       stationary broadcasts den to all partitions), rho = 1/den via
       reciprocal_approx_fast on DVE
    -> O^T = v^T @ P-hat^T on PE (v natural is already the right stationary)
    -> normalize O^T by rho + hardswish on DVE; v-bias of heads 3-5 folded
       here per-partition (softmax rows sum to 1, so O = P(xWv)/den + bv)
    -> A^T per head -> proj matmul (lagged one batch as PE filler) -> out.
  No transpose matmuls and no GpSimd work (the v1 kernel spent 70% of the
  span in GpSimd diag builds for a P^T-via-matmul transpose: 885us).

Optimizations over the 341us baseline (measures ~280us NTFF HW exec on a
cool device; the shared trn2 chip sometimes sits in a P0 2.0GHz state
where everything is uniformly ~19% slower):
  - proj computed transposed (out^T = W @ A^T) over batch PAIRS (N=392):
    3 full 128-row c-tiles instead of 128+68 token tiles, no 1-partition
    bias matmuls (proj bias rides the ACT evacuation per-partition),
    output [DIM, T] transposed back on host.
  - v-bias applied per-partition in O^T layout for ALL heads (vbc tensor
    _scalar adds); both v halves evacuate via ACT Copy.
  - q4/q5/k4/k5 packed into ONE qkv dest (3 full-width matmuls per chunk
    instead of 2 full + 2 half); k45 copied to rows 0:64 of a shadow tile
    by an SBUF->SBUF DMA (walrus requires lhsT/rhs partition bases match).
  - all 12 S^T matmuls emitted in one wave (4 tile_position row strips
    pipeline 3 matmuls each); ot_ps shares the sps PSUM slots (same
    shape+tag) so 3 s2 tiles + 2 mm banks fit the 8 PSUM banks.
  - xt split into 9 independently-DMA'd tiles, all inputs host-packed in
    exact SBUF layouts (straight large-descriptor DMA copies), ordered by
    first use; PE warmup matmuls trip the HAM clock gate during the
    initial DMA wait.
  - qkv chunks spread to batch bc-2 (QKV_AHEAD=1) so tail batches keep PE
    filler (prevents end-of-kernel HAM re-throttle); final pair's proj
    split per-batch to shorten the drain.
"""

import os
import sys

import numpy as np

sys.path.insert(0, "/opt/trn_rl_repo")

import ml_dtypes  # noqa: E402

import concourse.bass as bass  # noqa: E402
import concourse.tile as tile  # noqa: E402
from concourse import bacc, mybir  # noqa: E402
from concourse.bass_utils import run_bass_kernel_spmd  # noqa: E402

BF16 = mybir.dt.bfloat16
F32 = mybir.dt.float32
F8 = mybir.dt.float8e4
NPBF16 = ml_dtypes.bfloat16
NPF8 = mybir.dt.np(F8)
DR = mybir.MatmulPerfMode.DoubleRow

N_CORES = 8
B, N, DIM = 256, 196, 384
KD, NH, D = 32, 6, 128  # key dim, heads, per-head v dim
DH = D * NH  # 768
RES = 14
SCALE = KD ** -0.5

AF = mybir.ActivationFunctionType
OP = mybir.AluOpType

# per-batch key/query token tiling: 196 rows split 128 + 68
NT = [(0, 128), (128, 68)]

LAST_RESULT = {}  # test harness peeks at timing info here

# head-pairs whose attention bias is added into S on the PE (identity
# matmul accumulate) instead of multiplied as exp(bias) on the DVE —
# a balance knob between the two engines (0..3)
BADD_PAIRS = 0
# pack q4,q5,k4,k5 into ONE qkv dest tile (rows 0:32/32:64/64:96/96:128),
# saving 3 half-empty matmuls per chunk; S matmuls for heads 4-5 then use
# stationary/moving APs with different partition bases + explicit
# tile_position
QK_PACK = True


def _build_nc(bc):
    """Build the single-core program for bc batches (bc*196 tokens)."""
    T = bc * N
    assert T % 392 == 0
    nch = T // 392

    nc = bacc.Bacc("TRN2", target_bir_lowering=False, debug=False, num_devices=N_CORES)

    xt_d = nc.dram_tensor("xt", [8, 128, 3, T // 8], BF16, kind="ExternalInput")
    wqk_d = nc.dram_tensor("wqk", [128, 4, 3, 128], BF16, kind="ExternalInput")
    bqk_d = nc.dram_tensor("bqk", [128, 4], F32, kind="ExternalInput")
    wv_d = nc.dram_tensor("wv", [128, 3, DH], BF16, kind="ExternalInput")
    vb_d = nc.dram_tensor("vb", [128, DH], F32, kind="ExternalInput")
    vbc_d = nc.dram_tensor("vbc", [128, NH], F32, kind="ExternalInput")
    wpt_d = nc.dram_tensor("wpt", [128, NH, 3, 128], BF16, kind="ExternalInput")
    pbc_d = nc.dram_tensor("pbc", [128, 3], F32, kind="ExternalInput")
    onem_d = nc.dram_tensor("onem", [128, 128], BF16, kind="ExternalInput")
    ebt_d = nc.dram_tensor("ebt", [128, NH // 2, 1024], BF16, kind="ExternalInput")
    btt_d = nc.dram_tensor("btt", [128, NH // 2, 1024], BF16, kind="ExternalInput")
    idm_d = nc.dram_tensor("idm", [128, 128], BF16, kind="ExternalInput")
    out_d = nc.dram_tensor("out", [DIM, T], F32, kind="ExternalOutput")

    with tile.TileContext(nc) as tc:
        with (  # noqa: SIM117
            tc.tile_pool(name="const", bufs=1) as cpool,
            tc.tile_pool(name="qkt", bufs=1) as qpool,
            tc.tile_pool(name="vtile", bufs=4) as vpool,
            tc.tile_pool(name="pu", bufs=3) as pupool,
            tc.tile_pool(name="pb2", bufs=4) as pbpool,
            tc.tile_pool(name="rho", bufs=4) as rpool,
            tc.tile_pool(name="onm", bufs=3) as onpool,
            tc.tile_pool(name="atc", bufs=3) as atpool,
            tc.tile_pool(name="ob", bufs=3) as opool,
            tc.tile_pool(name="mmps", bufs=2, space="PSUM") as mmps,
            tc.tile_pool(name="sps", bufs=3, space="PSUM") as sps,
        ):
            # ---- constants into SBUF, ordered by first use so the first
            #      qkv chunk (wqk+xq0) isn't queued behind later-needed
            #      megabyte-scale DMAs. xt is split into 8 independent
            #      tiles (4 batches each) so early matmuls only wait on
            #      ~0.6MB of the 4.8MB input stream ----
            XQ = T // 8  # 784 tokens = 4 batches per xt tile
            # tile 0 split in half again so the very first qkv chunk (392
            # tokens) only waits on ~0.3MB
            xt_q = [
                cpool.tile([128, 3, 392], BF16, tag=f"xq0{i}", name=f"xq0{i}")
                for i in range(2)
            ] + [
                cpool.tile([128, 3, XQ], BF16, tag=f"xq{i}", name=f"xq{i}")
                for i in range(1, 8)
            ]
            # critical first-compute tensors split across BOTH DMA queues
            # so they land in parallel: wqk on sync, xq0a on scalar; onem
            # first (tiny) to feed the PE warmup matmuls below
            onem_t = cpool.tile([128, 128], BF16)
            nc.sync.dma_start(onem_t[:], onem_d[:])
            wqk_t = cpool.tile([128, 4, 3, 128], BF16)
            nc.sync.dma_start(wqk_t[:], wqk_d[:])
            nc.sync.dma_start(xt_q[0][:, :, :], xt_d[0, :, :, 0:392])
            nc.sync.dma_start(xt_q[1][:, :, :], xt_d[0, :, :, 392:784])
            bqk_t = cpool.tile([128, 4], F32)
            nc.sync.dma_start(bqk_t[:], bqk_d[:])
            wv_t = cpool.tile([128, 3, DH], BF16)
            nc.scalar.dma_start(wv_t[:], wv_d[:])
            vbc_t = cpool.tile([128, NH], F32)
            nc.sync.dma_start(vbc_t[:], vbc_d[:])
            ebt_t = cpool.tile([128, NH // 2, 1024], BF16)
            nc.scalar.dma_start(ebt_t[:], ebt_d[:])
            nc.sync.dma_start(xt_q[2][:, :, :], xt_d[1])
            if BADD_PAIRS:
                btt_t = cpool.tile([128, NH // 2, 1024], BF16)
                nc.scalar.dma_start(btt_t[:], btt_d[:])
                idm_t = cpool.tile([128, 128], BF16)
                nc.scalar.dma_start(idm_t[:], idm_d[:])
            pbc_t = cpool.tile([128, 3], F32)
            nc.scalar.dma_start(pbc_t[:], pbc_d[:])
            wpt_t = cpool.tile([128, NH, 3, 128], BF16)
            nc.scalar.dma_start(wpt_t[:], wpt_d[:])
            for i in range(2, 8):
                nc.sync.dma_start(xt_q[i + 1][:, :, :], xt_d[i])

            def xt_at(t0):
                """(tile, col offset) for global token t0 (range stays in-tile)."""
                if t0 < 784:
                    return xt_q[t0 // 392], t0 % 392
                return xt_q[2 + (t0 - 784) // XQ], (t0 - 784) % XQ

            # ~3.5us of dummy matmuls (onem x onem -> scratch PSUM) fill the
            # otherwise-idle PE during the initial input DMA, tripping the
            # HAM activity window so the first real matmuls run at 2.4GHz
            # instead of cold 1.2GHz. Rotating target regions lets them
            # pipeline back-to-back instead of serializing on WAW.
            warm_ps = mmps.tile([128, 392], F32, tag="mm", name="warm_ps")
            for i in range(32):
                r0 = 128 * (i % 3)
                nc.tensor.matmul(
                    warm_ps[:, r0:r0 + 128], onem_t[:, :], onem_t[:, :],
                    start=True, stop=True)

            # ---- q/k^T phase: tQ/tK hold q/k of heads 0-3 at partition 32h;
            #      heads 4-5 either packed in one tile (q4,q5,k4,k5 at rows
            #      0/32/64/96) or split across tQ2/tK2 (rows 0:64) ----
            tQ = qpool.tile([128, T], BF16, tag="tq")
            tK = qpool.tile([128, T], BF16, tag="tk")
            if QK_PACK:
                tQK2 = qpool.tile([128, T], BF16, tag="tqk2")
                # k4/k5 land at rows 64:128 of tQK2; a SBUF->SBUF DMA per
                # chunk copies them down to rows 0:64 of tK2c so the S
                # matmuls see matched q/k partition bases (walrus requires
                # lhsT.base == rhs.base)
                tK2c = qpool.tile([64, T], BF16, tag="tk2c")
                qk_dests = [(0, tQ, 128), (1, tK, 128), (2, tQK2, 128)]
            else:
                tQ2 = qpool.tile([128, T], BF16, tag="tq2")
                tK2 = qpool.tile([128, T], BF16, tag="tk2")
                qk_dests = [
                    (0, tQ, 128), (1, tK, 128), (2, tQ2, 64), (3, tK2, 64)]

            def emit_qkv_chunk(ch):
                c0 = 392 * ch
                xq, xo = xt_at(c0)
                for mt, dest, msz in qk_dests:
                    ps = mmps.tile([128, 392], F32, tag="mm")
                    for ct in range(3):
                        nc.tensor.matmul(
                            ps[0:msz, :],
                            wqk_t[:, mt, ct, 0:msz],
                            xq[:, ct, xo:xo + 392],
                            start=(ct == 0), stop=(ct == 2),
                        )
                    nc.scalar.activation(
                        dest[0:msz, c0:c0 + 392], ps[0:msz, :], AF.Identity,
                        bias=bqk_t[0:msz, mt:mt + 1], scale=1.0,
                    )
                if QK_PACK:
                    nc.sync.dma_start(
                        tK2c[0:64, c0:c0 + 392], tQK2[64:128, c0:c0 + 392])

            # chunk c feeds batches 2c and 2c+1; AHEAD chunks of runway up
            # front, the rest interleaved into the batch loop as PE filler.
            # AHEAD=1 keeps chunk emissions flowing through batch bc-2 so
            # the tail batches still have PE filler (prevents end-of-kernel
            # HAM re-throttle)
            QKV_AHEAD = 1
            for ch in range(QKV_AHEAD):
                emit_qkv_chunk(ch)

            def emit_s(hp, b0):
                """S^T matmuls for one head pair; head hh at column 512*hh of a
                bank-padded [128,1024] PSUM tile (m-tile mt at 512*hh+196*mt).
                Pairs 0 and 1 are emitted back-to-back so their 4 distinct
                tile_position row groups can run concurrently in the PE."""
                s2 = sps.tile([128, 1024], F32, tag="s")
                badd = hp < BADD_PAIRS
                if badd:
                    # seed the PSUM with the attention bias (id.T @ B^T = B^T);
                    # the S matmuls then accumulate on top (start=False)
                    for hh in range(2):
                        nc.tensor.matmul(
                            s2[:, 512 * hh:512 * hh + 392],
                            idm_t[:, :],
                            btt_t[:, hp, 512 * hh:512 * hh + 392],
                            start=True, stop=False,
                        )
                # mt-major order so consecutive matmuls hit different
                # tile_position row groups (they overlap in the array)
                for mt, (m0, msz) in enumerate(NT):
                    for hh in range(2):
                        h = 2 * hp + hh
                        if h < 4:
                            qb = kb = tp = 32 * h
                            qsrc = tQ
                            ksrc = tK
                        elif QK_PACK:
                            # q at rows 0:64 of tQK2, k copied to rows
                            # 0:64 of tK2c (matched partition bases)
                            qb = kb = tp = 32 * (h - 4)
                            qsrc, ksrc = tQK2, tK2c
                        else:
                            qb = kb = tp = 32 * (h - 4)
                            qsrc, ksrc = tQ2, tK2
                        nc.tensor.matmul(
                            s2[0:msz, 512 * hh + 196 * mt:512 * hh + 196 * mt + 196],
                            ksrc[kb:kb + 32, b0 + m0:b0 + m0 + msz],
                            qsrc[qb:qb + 32, b0:b0 + 196],
                            start=not badd, stop=True,
                            tile_position=(tp, 0),
                        )
                return s2

            def pair_exp(hp, s2):
                """ACT exp (+DVE bias-mul) for one head pair -> P-hat^T.
                Emitted for all three pairs right after the S wave so the
                exps sit at the FRONT of the ACT FIFO (they gate each
                pair's den/recip/OT chain; the v/filler evacuations can
                wait)."""
                s2v = s2[:].rearrange("p (h c) -> p h c", h=2)[:, :, 0:392]
                pb2 = pbpool.tile([128, 1024], BF16, tag="pb")
                pbv = pb2[:].rearrange("p (h c) -> p h c", h=2)[:, :, 0:392]
                if hp < BADD_PAIRS:
                    # bias already in S (PE-seeded): P-hat^T = exp(S^T+B^T)
                    nc.scalar.activation(pbv, s2v, AF.Exp)
                else:
                    # P-hat^T = exp(S^T) * exp(bias)^T
                    p_u = pupool.tile([128, 1024], BF16, tag="pu")
                    p_uv = p_u[:].rearrange("p (h c) -> p h c", h=2)[:, :, 0:392]
                    nc.scalar.activation(p_uv, s2v, AF.Exp)
                    ebv = ebt_t[:, hp, :].rearrange(
                        "p (h c) -> p h c", h=2)[:, :, 0:392]
                    nc.vector.tensor_mul(pbv, p_uv, ebv)
                return pb2

            def process_pair(hp, pb2, v_t, at2, par):
                # strided [128, 2, 392] views skip the 392:512 pad columns
                # (pads are never read downstream — den/OT use exact regions)
                # den[n] = sum_m P-hat^T[m, n] via all-ones matmul; the
                # [msz,128] ones stationary broadcasts den to all 128
                # partitions so the normalize multiply needs no
                # partition-broadcast AP (DVE requires nonzero step).
                # One matmul per m-tile covers BOTH heads via a strided
                # rhs AP [msz, 2, 196] (head stride 512).
                # den lands in ot_ps first; the O^T matmuls overwrite it
                # after the reciprocal is taken (WAR order via Tile).
                # ot_ps shares the sps pool slots (same shape+tag as s2) so
                # three s2 tiles can be alive at once within 8 PSUM banks;
                # only cols 0:392 are used.
                ot_ps = sps.tile([128, 1024], F32, tag="s", name="ot_ps")
                pb2v = pb2[:].rearrange("p (h c) -> p h c", h=2)
                for mt, (m0, msz) in enumerate(NT):
                    nc.tensor.matmul(
                        ot_ps[:, 0:392],
                        onem_t[0:msz, :],
                        pb2v[0:msz, :, 196 * mt:196 * mt + 196],
                        start=(mt == 0), stop=(mt == 1),
                    )
                rho = rpool.tile([128, 392], F32, tag="rho")
                nc.vector.reciprocal_approx_fast(rho[:, :], ot_ps[:, 0:392])
                # ---- O^T [128, 196] per head = v^T @ P-hat^T ----
                for hh in range(2):
                    h = 2 * hp + hh
                    for mt, (m0, msz) in enumerate(NT):
                        nc.tensor.matmul(
                            ot_ps[:, 196 * hh:196 * hh + 196],
                            v_t[0:msz, mt, 128 * h:128 * h + 128],
                            pb2[0:msz, 512 * hh + 196 * mt:512 * hh + 196 * mt + 196],
                            start=(mt == 0), stop=(mt == 1),
                        )
                # ---- normalize by rho; ALL heads add the v-bias here
                #      (per-partition in O^T layout; softmax rows sum to 1
                #      so O = P(xWv)/den + bv) ----
                o_n = onpool.tile([128, 392], BF16, tag="on")
                nc.vector.tensor_mul(o_n[:, :], ot_ps[:, 0:392], rho[:, :])
                o_nb = onpool.tile([128, 392], BF16, tag="onb")
                for hh in range(2):
                    h = 2 * hp + hh
                    reg = slice(196 * hh, 196 * hh + 196)
                    nc.vector.tensor_scalar_add(
                        o_nb[:, reg], o_n[:, reg], vbc_t[:, h:h + 1])
                # ---- 6*hardswish(O) = O*(clamp(O,-3,3)+3); the /6 is
                #      folded into the projection weights ----
                m_t = onpool.tile([128, 392], BF16, tag="mt")
                nc.vector.tensor_scalar(
                    out=m_t[:, :], in0=o_nb[:, :],
                    scalar1=3.0, scalar2=-3.0, op0=OP.min, op1=OP.max,
                )
                # at2 is [128, par, head, 196]; this write (par, 2hp..2hp+2)
                # is one contiguous [128, 392] span, keeping DVE in 2x mode
                # (scalar_tensor_tensor only runs at 1x — measured slower
                # than this TS+TT pair)
                m3 = onpool.tile([128, 392], BF16, tag="m3")
                nc.vector.tensor_scalar_add(m3[:, :], m_t[:, :], 3.0)
                nc.vector.tensor_mul(
                    at2[:, par, 2 * hp:2 * hp + 2, :], m3[:].rearrange(
                        "p (h c) -> p h c", h=2),
                    o_nb[:].rearrange("p (h c) -> p h c", h=2))

            def emit_proj(p, at2):
                # out^T [384, 392] for batch pair p: 3 ctiles x 6 head-ktiles,
                # proj bias added per-partition during the ACT evacuation
                c0 = 392 * p
                for ct in range(3):
                    ps = mmps.tile([128, 392], F32, tag="mm")
                    for kt in range(NH):
                        nc.tensor.matmul(
                            ps[:, :], wpt_t[:, kt, ct, :], at2[:, :, kt, :],
                            start=(kt == 0), stop=(kt == NH - 1),
                        )
                    ob = opool.tile([128, 392], F32, tag="ob")
                    nc.scalar.activation(
                        ob[:, :], ps[:, :], AF.Identity,
                        bias=pbc_t[:, ct:ct + 1], scale=1.0)
                    nc.sync.dma_start(
                        out_d[128 * ct:128 * ct + 128, c0:c0 + 392], ob[:, :])

            def emit_proj_half(p, at2, par):
                # one batch's worth of proj (N=196) — used to pull half of
                # the final pair's projection off the kernel tail
                c0 = 392 * p + 196 * par
                for ct in range(3):
                    ps = mmps.tile([128, 392], F32, tag="mm")
                    for kt in range(NH):
                        nc.tensor.matmul(
                            ps[:, 0:196], wpt_t[:, kt, ct, :],
                            at2[:, par, kt, :],
                            start=(kt == 0), stop=(kt == NH - 1),
                        )
                    ob = opool.tile([128, 392], F32, tag="ob")
                    nc.scalar.activation(
                        ob[:, 0:196], ps[:, 0:196], AF.Identity,
                        bias=pbc_t[:, ct:ct + 1], scale=1.0)
                    nc.sync.dma_start(
                        out_d[128 * ct:128 * ct + 128, c0:c0 + 196],
                        ob[:, 0:196])

            for b in range(bc):
                b0 = b * N
                # ---- v natural [tokens, 768] for this batch; low half
                #      evacuated on DVE (with bias add), high half on ACT
                #      (bias for heads 3-5 folded into o_n above) ----
                v_t = vpool.tile([128, 2, DH], BF16, tag="v")
                xq, xo = xt_at(b0)
                for nt, (r0, nsz) in enumerate(NT):
                    for half in range(2):
                        h0 = 384 * half
                        ps = mmps.tile([128, 392], F32, tag="mm")
                        for ct in range(3):
                            nc.tensor.matmul(
                                ps[0:nsz, 0:384],
                                xq[:, ct, xo + r0:xo + r0 + nsz],
                                wv_t[:, ct, h0:h0 + 384],
                                start=(ct == 0), stop=(ct == 2),
                            )
                        nc.scalar.activation(
                            v_t[0:nsz, nt, h0:h0 + 384], ps[0:nsz, 0:384],
                            AF.Copy)

                par = b % 2
                if par == 0:
                    if b > 0:
                        done_at2 = at2  # pair b//2-1 fully written
                    at2 = atpool.tile([128, 2, NH, 196], BF16, tag="at")
                # all 12 S matmuls in one wave: the 4 tile_position row
                # strips pipeline 3 matmuls each, and exp(a) starts as soon
                # as s2a's strip finishes
                s2a = emit_s(0, b0)
                s2b = emit_s(1, b0)
                s2c = emit_s(2, b0)
                # PE fillers while this batch's exp/bias-mul run: qkv chunks
                # on even batches, proj of the previous PAIR on odd batches
                if par == 0 and b // 2 + QKV_AHEAD < nch:
                    emit_qkv_chunk(b // 2 + QKV_AHEAD)
                if par == 1 and b > 1:
                    emit_proj(b // 2 - 1, done_at2)
                    if b == bc - 1:
                        # first half of the final pair's proj runs as
                        # filler here (its par=0 data completed last batch)
                        emit_proj_half(bc // 2 - 1, at2, 0)
                process_pair(0, pair_exp(0, s2a), v_t, at2, par)
                process_pair(1, pair_exp(1, s2b), v_t, at2, par)
                process_pair(2, pair_exp(2, s2c), v_t, at2, par)
            emit_proj_half(bc // 2 - 1, at2, 1)

    nc.finalize()  # run Bacc passes (reg alloc, wait splitting) before walrus
    return nc


def _host_pack(x, qkv_w, qkv_b, proj_w, proj_b, attn_biases, bias_idxs, bc):
    """Build the common (replicated) input map and per-core xt slices."""
    w = np.asarray(qkv_w, np.float32).reshape(NH, 192, DIM)
    bia = np.asarray(qkv_b, np.float32).reshape(NH, 192)
    qw = w[:, 0:KD, :] * SCALE          # [6, 32, 384]
    kw = w[:, KD:2 * KD, :]
    vw = w[:, 2 * KD:, :]               # [6, 128, 384]
    qb = bia[:, 0:KD] * SCALE
    kb = bia[:, KD:2 * KD]
    vb = bia[:, 2 * KD:]

    wqk = np.zeros((4, DIM, 128), np.float32)
    wqk[0, :, :] = qw[0:4].reshape(128, DIM).T
    wqk[1, :, :] = kw[0:4].reshape(128, DIM).T
    bqk = np.zeros((128, 4), np.float32)
    bqk[:, 0] = qb[0:4].reshape(128)
    bqk[:, 1] = kb[0:4].reshape(128)
    if QK_PACK:
        # q4,q5 at dest rows 0:64, k4,k5 at rows 64:128 of one tile
        wqk[2, :, 0:64] = qw[4:6].reshape(64, DIM).T
        wqk[2, :, 64:128] = kw[4:6].reshape(64, DIM).T
        bqk[0:64, 2] = qb[4:6].reshape(64)
        bqk[64:128, 2] = kb[4:6].reshape(64)
    else:
        wqk[2, :, 0:64] = qw[4:6].reshape(64, DIM).T
        wqk[3, :, 0:64] = kw[4:6].reshape(64, DIM).T
        bqk[0:64, 2] = qb[4:6].reshape(64)
        bqk[0:64, 3] = kb[4:6].reshape(64)

    wv = vw.reshape(DH, DIM).T.copy()          # [384, 768], head h at cols 128h
    vbt = np.tile(vb.reshape(1, DH), (128, 1)).astype(np.float32)
    vbc = np.ascontiguousarray(vb.T)           # [128, 6], col h = head h's bias
    # device computes 6*hardswish; absorb the 1/6 into the projection weights.
    # proj runs transposed (out^T = W @ A^T): wpt[p, kt, ct, c] = W6[128ct+c, 128kt+p]
    w6 = np.asarray(proj_w, np.float32) / 6.0  # [384, 768]
    wpt = np.ascontiguousarray(
        w6.reshape(3, 128, NH, 128).transpose(3, 2, 0, 1))  # [128, 6, 3, 128]
    pbc = np.ascontiguousarray(
        np.asarray(proj_b, np.float32).reshape(3, 128).T)  # [128, 3]

    bmat = np.asarray(attn_biases, np.float32)[:, np.asarray(bias_idxs)]  # [6,196,196]
    ebp = np.zeros((128, NH // 2, 1024), np.float32)  # exp(bias)^T for DVE pairs
    btp = np.zeros((128, NH // 2, 1024), np.float32)  # raw bias^T for PE-add pairs
    bT = np.transpose(bmat, (0, 2, 1))  # [6, m, n]
    ebT = np.exp(bT)
    for h in range(NH):
        hp, hh = divmod(h, 2)
        ebp[0:128, hp, 512 * hh + 0:512 * hh + 196] = ebT[h, 0:128, :]
        ebp[0:68, hp, 512 * hh + 196:512 * hh + 392] = ebT[h, 128:196, :]
        btp[0:128, hp, 512 * hh + 0:512 * hh + 196] = bT[h, 0:128, :]
        btp[0:68, hp, 512 * hh + 196:512 * hh + 392] = bT[h, 128:196, :]

    # DMA-friendly layouts: exactly the SBUF tile layouts, so every input
    # DMA is a straight large-descriptor copy (no gather)
    wqk_p = np.ascontiguousarray(
        wqk.reshape(4, 3, 128, 128).transpose(2, 0, 1, 3))  # [128,4,3,128]
    wv_p = np.ascontiguousarray(
        wv.reshape(3, 128, DH).transpose(1, 0, 2))  # [128,3,768]

    common = {
        "pbc": pbc,
        "btt": btp.astype(NPBF16),
        "idm": np.eye(128, dtype=np.float32).astype(NPBF16),
        "onem": np.ones((128, 128), NPBF16),
        "wqk": wqk_p.astype(NPBF16),
        "bqk": bqk,
        "wv": wv_p.astype(NPBF16),
        "vb": vbt,
        "vbc": vbc.astype(np.float32),
        "wpt": wpt.astype(NPBF16),
        "ebt": ebp.astype(NPBF16),
    }

    x = np.asarray(x, np.float32)
    n_cores = x.shape[0] // bc
    T = bc * N
    xts = []
    for c in range(n_cores):
        xc = x[bc * c:bc * (c + 1)].reshape(T, DIM)
        xt = np.ascontiguousarray(xc.T).astype(NPBF16)  # [384, T]
        xt8 = np.ascontiguousarray(
            xt.reshape(3, 128, 8, T // 8).transpose(2, 1, 0, 3))
        xts.append(xt8)  # [8, 128, 3, T//8]
    return common, xts


_NC_CACHE = {}


def kernel(x, qkv_w, qkv_b, proj_w, proj_b, attn_biases, bias_idxs):
    bc = B // N_CORES
    if bc not in _NC_CACHE:
        _NC_CACHE[bc] = _build_nc(bc)
    nc = _NC_CACHE[bc]
    common, xts = _host_pack(x, qkv_w, qkv_b, proj_w, proj_b, attn_biases, bias_idxs, bc)
    in_maps = [dict(common, xt=xts[c]) for c in range(N_CORES)]
    trace = bool(int(os.environ.get("KT_TRACE", "0")))
    res = run_bass_kernel_spmd(nc, in_maps, list(range(N_CORES)), trace=trace)
    LAST_RESULT["exec_time_ns"] = res.exec_time_ns
    LAST_RESULT["mean_exec_time_ns"] = res.mean_exec_time_ns
    # device emits out^T [DIM, T] per core; transpose back on host
    outs = [
        np.ascontiguousarray(res.results[c]["out"].T).reshape(bc, N, DIM)
        for c in range(N_CORES)
    ]
    return np.concatenate(outs, axis=0).astype(np.float32)

